# revision 7
# baseline (speedup 1.0000x reference)
"""MoE FeedForward kernel for Trainium2, expert-parallel across 8 NeuronCores.

Strategy (per sharding hint "expert parallel"):
  - Host computes the router + capacity-based dispatch in float64 numpy.
    This mirrors the reference computation (z -> sigmoid -> log -> softmax ->
    top-k -> per-expert capacity top-k) with higher precision than the f32
    reference, so the *selection* (which token goes to which expert) matches
    the reference's own f32 selection exactly (verified: min top-2 gap of the
    fixed input is 3e-6, f64 error ~1e-16, reference f32 error ~1e-7).
    The host then shards: core e receives its expert's gathered tokens
    (transposed, C-major) -- this realizes the "all-to-all the dispatched
    tokens" step -- plus replicated shared-FFN weights and its 1/8 token
    slice for the data-parallel shared FFN.
  - Device (one Bass/Tile program, SPMD on cores 0..7): dense expert FFN
    fc2(gelu(fc1(xe))) on [n_slots, C] gathered tokens + shared FFN
    sh2(gelu(sh1(xs))) on the core's [N/8, C] token slice.  All matmuls in
    float32r (full PE rate, ~1.5e-4 rel err).  Biases of the *first* layers
    are fused into the gelu activation on-device; second-layer biases are
    folded into the host combine (they commute with the weighted sum).
  - Host combines: y = sh + sh2_b + sum_k w_k * (oe[slot_k] + fc2_b[e_k]),
    reshaped to [B, T, C].

The device does 99.96% of the FLOPs (the two FFNs); the host does the
O(N*E*C) router matmul and the gathers/scatters that define the sharding.
"""
import math
from functools import lru_cache

import numpy as np

# ---------------- problem constants (hardcoded per the brief) -------------
B, T, C = 4, 2048, 1024
E, K = 8, 2
EH, SH = 2048, 4096
CAP_FACTOR = 1.25
SCALING = 1.0
FC_MULT = 1.0
N = B * T
N_CORES = 8
P = 128
TOKB = 256          # expert-stage token block (2 tok-tiles x 2 c-chunks = 4 psum banks)
CAP = max(int(math.ceil(math.ceil(N * K / E) * CAP_FACTOR)), 1)  # 2560

_DT = "float32r"    # matmul dtype for the big FFNs


# ---------------- host routing + dispatch (numpy, float64) ----------------
def host_route(x, proj_w, proj_b, expert_bias):
    """Mirror of the reference router+dispatch in float64.

    Returns dict with per-expert gather lists and per-token combine info.
    """
    xf = x.reshape(N, C).astype(np.float64)
    z = xf @ proj_w.astype(np.float64).T + proj_b.astype(np.float64)
    s = 1.0 / (1.0 + np.exp(-z)) + expert_bias.astype(np.float64)
    sc = np.clip(s, 1e-12, None)
    logits = np.log(sc) * SCALING
    # softmax (values only affect weights via ratios; selection via order)
    m = logits.max(axis=1, keepdims=True)
    p_un = np.exp(logits - m)
    probs = p_un / p_un.sum(axis=1, keepdims=True)

    order = np.argsort(-probs, axis=1, kind="stable")
    topk_idx = order[:, :K]                                   # [N, K]
    topk_p = np.take_along_axis(probs, topk_idx, axis=1)
    topk_w = topk_p / np.clip(topk_p.sum(1, keepdims=True), 1e-12, None)

    # pairs in reference order: p = t*K + k
    pair_e = topk_idx.reshape(-1)                             # [N*K]
    pair_w = topk_w.reshape(-1)
    pair_tok = np.repeat(np.arange(N), K)

    # per-expert capacity top-k by weight, ties -> lower pair index
    sort_idx = np.lexsort((np.arange(N * K), -pair_w, pair_e))
    e_sorted = pair_e[sort_idx]
    seg_start = np.searchsorted(e_sorted, np.arange(E))
    pos_in_e = np.arange(N * K) - seg_start[e_sorted]
    valid_sorted = pos_in_e < CAP

    counts = np.minimum(np.bincount(pair_e, minlength=E), CAP)
    max_count = int(counts.max())
    n_blocks = max(1, (max_count + TOKB - 1) // TOKB)
    n_slots = n_blocks * TOKB

    # forward map: kept_tok[e, pos] = token id (pad: token 0)
    kept_tok = np.zeros((E, n_slots), np.int64)
    kept_tok[e_sorted[valid_sorted], pos_in_e[valid_sorted]] = pair_tok[
        sort_idx[valid_sorted]
    ]
    # inverse map per pair
    slot_of_pair = np.full(N * K, 0, np.int64)
    pair_valid = np.zeros(N * K, bool)
    gslot = e_sorted * n_slots + pos_in_e
    slot_of_pair[sort_idx[valid_sorted]] = gslot[valid_sorted]
    pair_valid[sort_idx[valid_sorted]] = True

    return {
        "kept_tok": kept_tok,
        "slot": slot_of_pair.reshape(N, K),
        "valid": pair_valid.reshape(N, K),
        "w": topk_w,                      # float64 [N, K]
        "e": topk_idx,                    # [N, K]
        "n_slots": n_slots,
        "counts": counts,
    }


# ---------------- device program ------------------------------------------
@lru_cache(maxsize=4)
def build_module(n_slots, act="Gelu", stages="both"):
    import concourse.bacc as bacc
    import concourse.mybir as mybir
    from concourse.tile import TileContext

    fr = getattr(mybir.dt, _DT)
    f32 = mybir.dt.float32
    GELU = getattr(mybir.ActivationFunctionType, act)

    nc = bacc.Bacc("TRN2", target_bir_lowering=False)

    # inputs (per core)
    xeT_d = nc.dram_tensor("xeT", [C, n_slots], fr, kind="ExternalInput")
    fc1wT_d = nc.dram_tensor("fc1wT", [C, EH], fr, kind="ExternalInput")
    fc1b_d = nc.dram_tensor("fc1b", [P, EH // P], f32, kind="ExternalInput")
    fc2wT_d = nc.dram_tensor("fc2wT", [EH, C], fr, kind="ExternalInput")
    xsT_d = nc.dram_tensor("xsT", [C, N // N_CORES], fr, kind="ExternalInput")
    sh1wT_d = nc.dram_tensor("sh1wT", [C, SH], fr, kind="ExternalInput")
    sh1b_d = nc.dram_tensor("sh1b", [P, SH // P], f32, kind="ExternalInput")
    sh2wT_d = nc.dram_tensor("sh2wT", [SH, C], fr, kind="ExternalInput")
    # outputs
    oe_d = nc.dram_tensor("oe", [n_slots, C], f32, kind="ExternalOutput")
    sh_d = nc.dram_tensor("sh", [N // N_CORES, C], f32, kind="ExternalOutput")

    NB = n_slots // TOKB           # expert token blocks
    NTOK_S = N // N_CORES          # 1024 shared tokens per core
    CT = C // P                    # 8 contraction tiles over C
    ET = EH // P                   # 16 eh tiles
    ST = SH // P                   # 32 sh tiles
    SHH = ST // 2                  # 16 sh tiles per half

    with TileContext(nc) as tc:
        # ---------------- expert FFN ----------------
        if stages in ("both", "expert"):
         with tc.tile_pool(name="ew", bufs=1) as ew, \
             tc.tile_pool(name="exe", bufs=2) as exe, \
             tc.tile_pool(name="eh_sb", bufs=3) as eh_sb, \
             tc.tile_pool(name="eout", bufs=3) as eout, \
             tc.tile_pool(name="eps_h", bufs=2, space="PSUM") as eps_h, \
             tc.tile_pool(name="eps_o", bufs=6, space="PSUM") as eps_o:

            fc1w_sb = ew.tile([P, CT * EH], fr)      # [p, ct*EH + m]
            fc2w_sb = ew.tile([P, ET * C], fr)       # [p, et*C + m]
            fc1b_sb = ew.tile([P, ET], f32)
            nc.sync.dma_start(fc1b_sb[:], fc1b_d[:])
            # split weight loads per eh-strip so early tiles arrive fast
            for et in range(ET):
                nc.sync.dma_start(
                    fc1w_sb[:, :].rearrange("p (ct m) -> p ct m", ct=CT)[
                        :, :, et * P:(et + 1) * P
                    ],
                    fc1wT_d[:, et * P:(et + 1) * P].rearrange(
                        "(ct p) m -> p ct m", p=P
                    ),
                )
                nc.sync.dma_start(
                    fc2w_sb[:, et * C:(et + 1) * C],
                    fc2wT_d[et * P:(et + 1) * P, :],
                )

            for b in range(NB):
                xe_t = exe.tile([P, CT * TOKB], fr, tag="xe")
                nc.sync.dma_start(
                    xe_t[:].rearrange("p (ct n) -> p ct n", ct=CT),
                    xeT_d[:, b * TOKB:(b + 1) * TOKB].rearrange(
                        "(ct p) n -> p ct n", p=P
                    ),
                )
                accs = [
                    eps_o.tile([P, 512], f32, space="PSUM", tag="acc",
                               name=f"acc_{b}_{i}")
                    for i in range(4)
                ]
                for et in range(ET):
                    ps_h = eps_h.tile([P, TOKB], f32, space="PSUM", tag="h")
                    for ct in range(CT):
                        nc.tensor.matmul(
                            ps_h[:],
                            lhsT=fc1w_sb[:, ct * EH + et * P: ct * EH + (et + 1) * P],
                            rhs=xe_t[:, ct * TOKB:(ct + 1) * TOKB],
                            start=(ct == 0), stop=(ct == CT - 1),
                        )
                    h_t = eh_sb.tile([P, TOKB], fr, tag="h_sb")
                    nc.scalar.activation(
                        h_t[:], ps_h[:], GELU, bias=fc1b_sb[:, et:et + 1]
                    )
                    for tt in range(TOKB // P):
                        for cc in range(2):
                            nc.tensor.matmul(
                                accs[tt * 2 + cc][:],
                                lhsT=h_t[:, tt * P:(tt + 1) * P],
                                rhs=fc2w_sb[:, et * C + cc * 512: et * C + (cc + 1) * 512],
                                start=(et == 0), stop=(et == ET - 1),
                            )
                for tt in range(TOKB // P):
                    for cc in range(2):
                        o_t = eout.tile([P, 512], f32, tag="o")
                        nc.scalar.copy(o_t[:], accs[tt * 2 + cc][:])
                        nc.sync.dma_start(
                            oe_d[b * TOKB + tt * P: b * TOKB + (tt + 1) * P,
                                 cc * 512:(cc + 1) * 512],
                            o_t[:],
                        )

        # ---------------- shared FFN (SH split in quarters) ----------------
        NSPLIT = 4
        SHQ = ST // NSPLIT         # 8 sh-tiles per quarter
        if stages in ("both", "shared"):
         with tc.tile_pool(name="sxs", bufs=1) as sxs, \
             tc.tile_pool(name="sw1", bufs=2) as sw1, \
             tc.tile_pool(name="sw2", bufs=1) as sw2, \
             tc.tile_pool(name="shs", bufs=1) as shs, \
             tc.tile_pool(name="sout", bufs=1) as sout, \
             tc.tile_pool(name="so_st", bufs=3) as so_st, \
             tc.tile_pool(name="sps_h", bufs=2, space="PSUM") as sps_h, \
             tc.tile_pool(name="sps_o", bufs=3, space="PSUM") as sps_o:

            xs_sb = sxs.tile([P, CT * NTOK_S], fr)
            nc.sync.dma_start(
                xs_sb[:].rearrange("p (ct n) -> p ct n", ct=CT),
                xsT_d[:].rearrange("(ct p) n -> p ct n", p=P),
            )
            sh1b_sb = sxs.tile([P, ST], f32)
            nc.sync.dma_start(sh1b_sb[:], sh1b_d[:])
            out_sb = sout.tile([P, (NTOK_S // P) * C], f32)   # [p, tt*C + c]

            for q in range(NSPLIT):
                hs_q = shs.tile([P, SHQ * NTOK_S], fr, tag="hs")
                w2_q = sw2.tile([P, SHQ * C], fr, tag="w2")
                nc.sync.dma_start(
                    w2_q[:].rearrange("p (s m) -> p s m", s=SHQ),
                    sh2wT_d[q * SHQ * P:(q + 1) * SHQ * P, :].rearrange(
                        "(s p) m -> p s m", p=P
                    ),
                )
                for sti in range(SHQ):
                    st = q * SHQ + sti
                    w1_t = sw1.tile([P, CT * P], fr, tag="w1")
                    nc.sync.dma_start(
                        w1_t[:].rearrange("p (ct m) -> p ct m", ct=CT),
                        sh1wT_d[:, st * P:(st + 1) * P].rearrange(
                            "(ct p) m -> p ct m", p=P
                        ),
                    )
                    for nb in range(NTOK_S // 512):
                        ps_h = sps_h.tile([P, 512], f32, space="PSUM", tag="sh_h")
                        for ct in range(CT):
                            nc.tensor.matmul(
                                ps_h[:],
                                lhsT=w1_t[:, ct * P:(ct + 1) * P],
                                rhs=xs_sb[:, ct * NTOK_S + nb * 512:
                                          ct * NTOK_S + (nb + 1) * 512],
                                start=(ct == 0), stop=(ct == CT - 1),
                            )
                        nc.scalar.activation(
                            hs_q[:, sti * NTOK_S + nb * 512:
                                 sti * NTOK_S + (nb + 1) * 512],
                            ps_h[:], GELU, bias=sh1b_sb[:, st:st + 1],
                        )
                for tt in range(NTOK_S // P):
                    for cc in range(2):
                        ps_o = sps_o.tile([P, 512], f32, space="PSUM", tag="sh_o")
                        for sti in range(SHQ):
                            nc.tensor.matmul(
                                ps_o[:],
                                lhsT=hs_q[:, sti * NTOK_S + tt * P:
                                          sti * NTOK_S + (tt + 1) * P],
                                rhs=w2_q[:, sti * C + cc * 512:
                                         sti * C + (cc + 1) * 512],
                                start=(sti == 0), stop=(sti == SHQ - 1),
                            )
                        if q == 0:
                            nc.scalar.copy(
                                out_sb[:, tt * C + cc * 512: tt * C + (cc + 1) * 512],
                                ps_o[:],
                            )
                        else:
                            nc.vector.tensor_add(
                                out_sb[:, tt * C + cc * 512: tt * C + (cc + 1) * 512],
                                out_sb[:, tt * C + cc * 512: tt * C + (cc + 1) * 512],
                                ps_o[:],
                            )
            for tt in range(NTOK_S // P):
                st_t = so_st.tile([P, C], f32, tag="st")
                nc.vector.tensor_copy(st_t[:], out_sb[:, tt * C:(tt + 1) * C])
                nc.sync.dma_start(sh_d[tt * P:(tt + 1) * P, :], st_t[:])

    nc.compile()
    return nc


# ---------------- host <-> device glue ------------------------------------
def make_in_maps(inputs, route):
    x = np.ascontiguousarray(np.asarray(inputs["x"], np.float32).reshape(N, C))
    xT = np.ascontiguousarray(x.T)
    n_slots = route["n_slots"]
    kept_tok = route["kept_tok"]

    fc1_w = np.asarray(inputs["fc1_w"], np.float32)
    fc2_w = np.asarray(inputs["fc2_w"], np.float32)
    fc1_b = np.asarray(inputs["fc1_b"], np.float32)
    sh1_w = np.asarray(inputs["sh1_w"], np.float32)
    sh2_w = np.asarray(inputs["sh2_w"], np.float32)
    sh1_b = np.asarray(inputs["sh1_b"], np.float32)

    sh1wT = np.ascontiguousarray(sh1_w.T)                   # [C, SH]
    sh2wT = np.ascontiguousarray(sh2_w.T)                   # [SH, C]
    sh1b_r = np.ascontiguousarray(sh1_b.reshape(SH // P, P).T)  # [P, ST]

    in_maps = []
    for e in range(N_CORES):
        xeT = np.ascontiguousarray(xT[:, kept_tok[e]])      # [C, n_slots]
        in_maps.append({
            "xeT": xeT,
            "fc1wT": np.ascontiguousarray(fc1_w[e].T),      # [C, EH]
            "fc1b": np.ascontiguousarray(fc1_b[e].reshape(EH // P, P).T),
            "fc2wT": np.ascontiguousarray(fc2_w[e].T),      # [EH, C]
            "xsT": np.ascontiguousarray(
                xT[:, e * (N // N_CORES):(e + 1) * (N // N_CORES)]),
            "sh1wT": sh1wT,
            "sh1b": sh1b_r,
            "sh2wT": sh2wT,
        })
    return in_maps


def combine(inputs, route, oe_list, sh_list):
    n_slots = route["n_slots"]
    O_all = np.concatenate(oe_list, axis=0)                 # [E*n_slots, C]
    sh_full = np.concatenate(sh_list, axis=0)               # [N, C]

    fc2_b = np.asarray(inputs["fc2_b"], np.float32)         # [E, C]
    sh2_b = np.asarray(inputs["sh2_b"], np.float32)         # [C]

    w = route["w"].astype(np.float32)                       # [N, K]
    valid = route["valid"]
    slot = route["slot"]
    e_idx = route["e"]

    y = sh_full + sh2_b[None, :]
    for k in range(K):
        wk = np.where(valid[:, k], w[:, k], 0.0).astype(np.float32)[:, None]
        y = y + wk * (O_all[slot[:, k]] + fc2_b[e_idx[:, k]])
    return (y * FC_MULT).reshape(B, T, C).astype(np.float32)


def kernel(**inputs) -> np.ndarray:
    from concourse.bass_utils import run_bass_kernel_spmd

    x = np.asarray(inputs["x"], np.float32)
    route = host_route(
        x.reshape(N, C),
        np.asarray(inputs["proj_w"], np.float32),
        np.asarray(inputs["proj_b"], np.float32),
        np.asarray(inputs["expert_bias"], np.float32),
    )
    nc = build_module(route["n_slots"])
    in_maps = make_in_maps(inputs, route)
    res = run_bass_kernel_spmd(nc, in_maps, core_ids=list(range(N_CORES)))
    oe_list = [res.results[e]["oe"] for e in range(N_CORES)]
    sh_list = [res.results[e]["sh"] for e in range(N_CORES)]
    return combine(inputs, route, oe_list, sh_list)
